# revision 1
# baseline (speedup 1.0000x reference)
"""Trainium2 Bass kernel for nn_AttnBlock_Spatio_Temporal (B=4,T=5,C=512,H=W=32).

Distribution: 8 cores = (video b in 0..3) x (pixel-half h in 0..1).
Host rolls the HW axis per core so its own 512 pixels come first (spatial
attention / GroupNorm are permutation-invariant over key pixels). Each core
computes full-frame k/v but only its own queries; the temporal GroupNorm
needs full-frame stats -> tiny per-frame pair AllReduce of per-channel
(sum, sumsq). All heavy matmuls run in bf16 (fp32 accumulate); residual adds
stay fp32.

Channel-major layout: channel c lives at (partition p, block j) with
c = 4p + j, so DRAM rows stream in 16KB-contiguous runs (4x fewer DMA
descriptors) and GroupNorm groups (16 consecutive channels) = 4 consecutive
partitions. Weight matrices get their columns host-permuted to keep conv
outputs in the same convention with contiguous lhsT slices.
"""
import numpy as np

B, T, C, HW = 4, 5, 512, 1024
G = 32
EPS = 1e-6
P = 128
CB = C // P          # 4 channel blocks
HALF = HW // 2       # 512 own pixels
KB = HW // P         # 8 key-pixel blocks
QB = HALF // P       # 4 query/pixel blocks
SCALE = float(C) ** -0.5
INV_CNT = 1.0 / 16384.0   # per-group element count (16ch*1024px or 16ch*512px*2)

_CACHE = {}


def _build():
    import concourse.bacc as bacc
    import concourse.tile as tile
    import concourse.mybir as mybir

    f32 = mybir.dt.float32
    bf16 = mybir.dt.bfloat16
    MULT = mybir.AluOpType.mult
    ADD = mybir.AluOpType.add
    SUB = mybir.AluOpType.subtract
    AF = mybir.ActivationFunctionType
    AX = mybir.AxisListType

    nc = bacc.Bacc("TRN2", target_bir_lowering=False, debug=False, num_devices=8)

    x_d = nc.dram_tensor("x", [T, C, HW], f32, kind="ExternalInput").ap()
    w_names = ["wq", "wk", "wv", "wo", "wqt", "wkt", "wvt", "wot"]
    w_d = {nm: nc.dram_tensor(nm + "T", [C, C], bf16, kind="ExternalInput").ap()
           for nm in w_names}
    b_d = {nm: nc.dram_tensor(nm, [C], f32, kind="ExternalInput").ap()
           for nm in ["bq", "bk", "bv", "bo", "bot"]}
    bqt_d = nc.dram_tensor("bqt", [C], bf16, kind="ExternalInput").ap()
    g_d = {nm: nc.dram_tensor(nm, [C], f32, kind="ExternalInput").ap()
           for nm in ["gamma_s", "beta_s", "gamma_t", "beta_t"]}
    sel_d = nc.dram_tensor("sel4", [P, G], f32, kind="ExternalInput").ap()
    bc_d = nc.dram_tensor("bcast4", [G, P], f32, kind="ExternalInput").ap()
    out_d = nc.dram_tensor("out", [T, C, HALF], f32, kind="ExternalOutput").ap()

    def cpart(ap_1d):  # [C] dram -> [128, CB] tile order (c = p*CB + j)
        return ap_1d.rearrange("(p j) -> p j", p=P)

    with tile.TileContext(nc) as tc:
        with tc.tile_pool(name="consts", bufs=1) as consts, \
             tc.tile_pool(name="stat4", bufs=4) as stat4, \
             tc.tile_pool(name="spatio_p", bufs=T) as spatio_p, \
             tc.tile_pool(name="psum", bufs=6, space="PSUM") as psum, \
             tc.tile_pool(name="ps_aff", bufs=2, space="PSUM") as ps_aff, \
             tc.tile_pool(name="dram", bufs=T, space="DRAM") as dram:

            # ---------------- constants ----------------
            w_sb = {}
            for nm in w_names:
                w_sb[nm] = consts.tile([P, CB, C], bf16, tag="w_" + nm,
                                       name="w_" + nm)
            bias_sb = {}
            for nm in ["bq", "bk", "bv", "bo", "bot"]:
                bias_sb[nm] = consts.tile([P, CB], f32, tag="b_" + nm,
                                          name="b_" + nm)
                nc.sync.dma_start(out=bias_sb[nm], in_=cpart(b_d[nm]))
            gam_sb = {}
            for nm in ["gamma_s", "beta_s", "gamma_t", "beta_t"]:
                gam_sb[nm] = consts.tile([P, CB], f32, tag="g_" + nm,
                                         name="g_" + nm)
                nc.sync.dma_start(out=gam_sb[nm], in_=cpart(g_d[nm]))
            bqt_bc = consts.tile([P, C], bf16, tag="bqt_bc", name="bqt_bc")
            nc.sync.dma_start(out=bqt_bc, in_=bqt_d.unsqueeze(0).to_broadcast([P, C]))
            sel4 = consts.tile([P, G], f32, tag="sel4", name="sel4")
            nc.sync.dma_start(out=sel4, in_=sel_d)
            bcast4 = consts.tile([G, P], f32, tag="bcast4", name="bcast4")
            nc.sync.dma_start(out=bcast4, in_=bc_d)
            eps32 = consts.tile([G, 1], f32, tag="eps32", name="eps32")
            nc.vector.memset(eps32, EPS)

            def affine_core(g2, gamma, beta, scale_out, shift_out):
                """g2: [G, 2] group (sum, sumsq); writes scale/shift [P, CB]."""
                m2 = stat4.tile([G, 2], f32, tag="m2", name="m2")
                nc.scalar.activation(out=m2, in_=g2, func=AF.Copy, scale=INV_CNT)
                rg = stat4.tile([G, 2], f32, tag="rg", name="rg")
                nc.vector.tensor_tensor(out=rg[:, 0:1], in0=m2[:, 0:1],
                                        in1=m2[:, 0:1], op=MULT)
                nc.vector.tensor_tensor(out=rg[:, 0:1], in0=m2[:, 1:2],
                                        in1=rg[:, 0:1], op=SUB)
                nc.scalar.activation(out=rg[:, 0:1], in_=rg[:, 0:1],
                                     func=AF.Sqrt, bias=eps32, scale=1.0)
                nc.vector.reciprocal(rg[:, 0:1], rg[:, 0:1])
                # rg[:,1] = -mean * rstd
                nc.vector.tensor_scalar(out=rg[:, 1:2], in0=m2[:, 0:1],
                                        scalar1=rg[:, 0:1], scalar2=-1.0,
                                        op0=MULT, op1=MULT)
                ps_bc = ps_aff.tile([P, 512], f32, tag="ps_stat", name="ps_bc")
                nc.tensor.matmul(ps_bc[:, 0:2], bcast4[:, :], rg[:, :],
                                 start=True, stop=True)
                nc.vector.tensor_scalar_mul(out=scale_out, in0=gamma,
                                            scalar1=ps_bc[:, 0:1])
                nc.vector.scalar_tensor_tensor(out=shift_out, in0=gamma,
                                               scalar=ps_bc[:, 1:2], in1=beta,
                                               op0=MULT, op1=ADD)

            gnt = [None] * T
            spatio_tiles = [None] * T
            bounce_outs = [None] * T

            # ================= spatial phase =================
            with tc.tile_pool(name="spat2", bufs=2) as spat2, \
                 tc.tile_pool(name="spat1", bufs=1) as spat1:

                def temporal_tail(fi):
                    gsum = stat4.tile([G, 2], f32, tag="gsum", name="gsum")
                    nc.sync.dma_start(out=gsum[:], in_=bounce_outs[fi][:])
                    scale_t = stat4.tile([P, CB], f32, tag="scale_t",
                                         name="scale_t")
                    shift_t = stat4.tile([P, CB], f32, tag="shift_t",
                                         name="shift_t")
                    affine_core(gsum, gam_sb["gamma_t"], gam_sb["beta_t"],
                                scale_t, shift_t)
                    gnt[fi] = spatio_p.tile([P, CB, HALF], bf16, tag="gnt",
                                            name="gnt")
                    for j in range(CB):
                        nc.vector.tensor_scalar(
                            out=gnt[fi][:, j, :], in0=spatio_tiles[fi][:, j, :],
                            scalar1=scale_t[:, j:j + 1],
                            scalar2=shift_t[:, j:j + 1],
                            op0=MULT, op1=ADD)

                xfs = [None] * T
                hns = [None] * T

                def load_x(fi):
                    xf = spat2.tile([P, CB, HW], f32, tag="xf", name="xf")
                    nc.sync.dma_start(
                        out=xf, in_=x_d[fi].rearrange("(p j) hw -> p j hw", p=P))
                    xfs[fi] = xf

                def gn_block(fi):
                    if xfs[fi] is None:
                        load_x(fi)
                    xf = xfs[fi]
                    sums = spat2.tile([P, CB, 2], f32, tag="sums", name="sums")
                    sqj = spat2.tile([P, HW], f32, tag="sqj", name="sqj")
                    for j in range(CB):
                        nc.vector.tensor_reduce(out=sums[:, j, 0:1], in_=xf[:, j, :],
                                                axis=AX.X, op=ADD)
                        nc.scalar.activation(out=sqj, in_=xf[:, j, :],
                                             func=AF.Square,
                                             accum_out=sums[:, j, 1:2])
                    ps_g = ps_aff.tile([P, 512], f32, tag="ps_stat", name="ps_g")
                    nc.tensor.matmul(ps_g[0:G, 0:2 * CB], sel4[:, :],
                                     sums.rearrange("p j s -> p (j s)"),
                                     start=True, stop=True)
                    g2s = stat4.tile([G, 2], f32, tag="g2s", name="g2s")
                    nc.vector.tensor_reduce(
                        out=g2s,
                        in_=ps_g[0:G, 0:2 * CB].rearrange("g (j s) -> g s j", s=2),
                        axis=AX.X, op=ADD)
                    scale_s = stat4.tile([P, CB], f32, tag="scale_s", name="scale_s")
                    shift_s = stat4.tile([P, CB], f32, tag="shift_s", name="shift_s")
                    affine_core(g2s, gam_sb["gamma_s"], gam_sb["beta_s"],
                                scale_s, shift_s)
                    hn = spat2.tile([P, CB, HW], bf16, tag="hn", name="hn")
                    for j in range(CB):
                        nc.vector.tensor_scalar(
                            out=hn[:, j, :], in0=xf[:, j, :],
                            scalar1=scale_s[:, j:j + 1], scalar2=shift_s[:, j:j + 1],
                            op0=MULT, op1=ADD)
                    hns[fi] = hn

                gn_block(0)
                for nm in ["wk", "wq", "wv", "wo", "wqt", "wkt", "wvt", "wot"]:
                    nc.sync.dma_start(
                        out=w_sb[nm],
                        in_=w_d[nm].rearrange("(p kc) co -> p kc co", p=P))
                ks = [None] * T
                qs = [None] * T
                vs = [None] * T

                def conv_block(fi):
                    hnl = hns[fi]
                    k_sb = spat1.tile([P, CB, HW], bf16, tag="k_sb", name="k_sb",
                                      bufs=2)
                    for jo in range(CB):
                        for half in range(2):
                            ps = psum.tile([P, 512], f32, tag="psc", name="psc")
                            for kc in range(CB):
                                nc.tensor.matmul(
                                    ps[:, :], w_sb["wk"][:, kc, jo * P:(jo + 1) * P],
                                    hnl[:, kc, half * 512:(half + 1) * 512],
                                    start=(kc == 0), stop=(kc == CB - 1))
                            nc.vector.tensor_scalar_add(
                                out=k_sb[:, jo, half * 512:(half + 1) * 512],
                                in0=ps, scalar1=bias_sb["bk"][:, jo:jo + 1])
                    q_sb = spat1.tile([P, CB, HALF], bf16, tag="q_sb", name="q_sb",
                                      bufs=2)
                    for jo in range(CB):
                        ps = psum.tile([P, 512], f32, tag="psc", name="psc")
                        for kc in range(CB):
                            nc.tensor.matmul(
                                ps[:, :], w_sb["wq"][:, kc, jo * P:(jo + 1) * P],
                                hnl[:, kc, 0:HALF],
                                start=(kc == 0), stop=(kc == CB - 1))
                        nc.vector.tensor_scalar_add(
                            out=q_sb[:, jo, :], in0=ps,
                            scalar1=bias_sb["bq"][:, jo:jo + 1])
                    vT_sb = spat1.tile([P, KB, C], bf16, tag="vT_sb", name="vT_sb",
                                       bufs=2)
                    for pb in range(KB):
                        ps = psum.tile([P, 512], f32, tag="psc", name="psc")
                        for kc in range(CB):
                            nc.tensor.matmul(
                                ps[:, :], hnl[:, kc, pb * P:(pb + 1) * P],
                                w_sb["wv"][:, kc, :],
                                start=(kc == 0), stop=(kc == CB - 1))
                        nc.scalar.copy(out=vT_sb[:, pb, :], in_=ps)
                    ks[fi], qs[fi], vs[fi] = k_sb, q_sb, vT_sb

                conv_block(0)
                for f in range(T):
                    xf = xfs[f]
                    k_sb, q_sb, vT_sb = ks[f], qs[f], vs[f]
                    if f + 1 < T:
                        load_x(f + 1)

                    if f + 1 < T:
                        gn_block(f + 1)

                    # ---- scores + softmax (no max-subtraction: |scores| ~ 1) ----
                    att = spat1.tile([P, QB, HW], bf16, tag="att", name="att")
                    den = spat2.tile([P, QB, 2], f32, tag="den", name="den")
                    for qb in range(QB):
                        for half in range(2):
                            psS = psum.tile([P, 512], f32, tag="psc", name="psc")
                            for kc in range(CB):
                                nc.tensor.matmul(
                                    psS[:, :],
                                    q_sb[:, kc, qb * P:(qb + 1) * P],
                                    k_sb[:, kc, half * 512:(half + 1) * 512],
                                    start=(kc == 0), stop=(kc == CB - 1))
                            nc.scalar.activation(
                                out=att[:, qb, half * 512:(half + 1) * 512],
                                in_=psS, func=AF.Exp, scale=SCALE,
                                accum_out=den[:, qb, half:half + 1])
                    rden = spat2.tile([P, QB], f32, tag="rden", name="rden")
                    nc.vector.tensor_reduce(out=rden, in_=den, axis=AX.X, op=ADD)
                    nc.vector.reciprocal(rden, rden)
                    for qb in range(QB):
                        nc.gpsimd.tensor_tensor(
                            out=att[:, qb, :], in0=att[:, qb, :],
                            in1=rden[:, qb:qb + 1].to_broadcast([P, HW]), op=MULT)

                    # ---- transpose att (batched DMA xbar) -> attT[kpix, kb, q] ----
                    attT = spat1.tile([P, KB, HALF], bf16, tag="attT", name="attT", bufs=2)
                    for qb in range(QB):
                        nc.sync.dma_start(
                            out=attT[:, :, qb * P:(qb + 1) * P],
                            in_=att[:, qb, :], transpose=True)

                    # next frame's convs: PE filler under the
                    # att-transpose latency
                    if f + 1 < T:
                        conv_block(f + 1)

                    # ---- hsp = v @ attT  (+bv via softmax-sums-to-1) ----
                    hsp = spat1.tile([P, CB, HALF], bf16, tag="hsp", name="hsp")
                    for cb in range(CB):
                        ps = psum.tile([P, 512], f32, tag="psc", name="psc")
                        for kb in range(KB):
                            nc.tensor.matmul(
                                ps[:, :], vT_sb[:, kb, cb * P:(cb + 1) * P],
                                attT[:, kb, :],
                                start=(kb == 0), stop=(kb == KB - 1))
                        nc.scalar.activation(
                            out=hsp[:, cb, :], in_=ps, func=AF.Identity,
                            bias=bias_sb["bv"][:, cb:cb + 1])

                    # ---- spatio = x + wo @ hsp + bo ; GN_t partial sums ----
                    spatio = spatio_p.tile([P, CB, HALF], bf16, tag="spatio",
                                           name="spatio")
                    sums_t = spat2.tile([P, CB, 2], f32, tag="sums_t", name="sums_t")
                    sqt = spat2.tile([P, 512], f32, tag="sqt", name="sqt")
                    for cb in range(CB):
                        ps = psum.tile([P, 512], f32, tag="psc", name="psc")
                        for kc in range(CB):
                            nc.tensor.matmul(
                                ps[:, :], w_sb["wo"][:, kc, cb * P:(cb + 1) * P],
                                hsp[:, kc, :],
                                start=(kc == 0), stop=(kc == CB - 1))
                        tmpo = spat2.tile([P, 512], f32, tag="tmpo", name="tmpo")
                        nc.scalar.activation(out=tmpo, in_=ps, func=AF.Identity,
                                             bias=bias_sb["bo"][:, cb:cb + 1])
                        nc.gpsimd.tensor_tensor(out=spatio[:, cb, :], in0=tmpo,
                                                in1=xf[:, cb, 0:HALF], op=ADD)
                        nc.vector.tensor_reduce(out=sums_t[:, cb, 0:1],
                                                in_=spatio[:, cb, :],
                                                axis=AX.X, op=ADD)
                        nc.scalar.activation(out=sqt, in_=spatio[:, cb, :],
                                             func=AF.Square,
                                             accum_out=sums_t[:, cb, 1:2])

                    # ---- per-frame pair AllReduce of GN_t sums -> gnt[f] ----
                    ps_gt = ps_aff.tile([P, 512], f32, tag="ps_stat", name="ps_gt")
                    nc.tensor.matmul(ps_gt[0:G, 0:2 * CB], sel4[:, :],
                                     sums_t.rearrange("p j s -> p (j s)"),
                                     start=True, stop=True)
                    g2t = stat4.tile([G, 2], f32, tag="g2t", name="g2t")
                    nc.vector.tensor_reduce(
                        out=g2t,
                        in_=ps_gt[0:G, 0:2 * CB].rearrange("g (j s) -> g s j", s=2),
                        axis=AX.X, op=ADD)
                    bounce_in = dram.tile([G, 2], f32, tag="bnc_in", name="bnc_in")
                    bounce_outs[f] = dram.tile([G, 2], f32, tag="bnc_out",
                                               name="bnc_out")
                    nc.sync.dma_start(out=bounce_in[:], in_=g2t[:])
                    nc.gpsimd.collective_compute(
                        "AllReduce", ADD,
                        replica_groups=[[0, 1], [2, 3], [4, 5], [6, 7]],
                        ins=[bounce_in.opt()], outs=[bounce_outs[f].opt()])
                    spatio_tiles[f] = spatio
                    # frame f-1's post-collective tail (one frame of slack so
                    # the in-order engine queues never wait on the collective)
                    if f > 0:
                        temporal_tail(f - 1)
                if True:
                    temporal_tail(T - 1)

            # ================= temporal phase =================
            # Per pixel-block pb: pack q,k as [P, T, C] and v as [P, C, T],
            # then per-pixel 5x5 attention via batched mul+reduce on DVE.
            with tc.tile_pool(name="temp5", bufs=T) as temp5, \
                 tc.tile_pool(name="temp2", bufs=2) as temp2, \
                 tc.tile_pool(name="temp4", bufs=4) as temp4:
                htp_b = []
                for t in range(T):
                    htp_b.append(temp5.tile([P, QB, C], bf16, tag="htp_b",
                                            name="htp_b"))
                for pb in range(QB):
                    qp = temp2.tile([P, T, C], bf16, tag="q5P", name="q5P")
                    kp = temp2.tile([P, T, C], bf16, tag="k5P", name="k5P")
                    vp = temp2.tile([P, T, C], bf16, tag="v5P", name="v5P")
                    for t in range(T):
                        for w_nm, dst in (("wqt", qp[:, t, :]), ("wkt", kp[:, t, :]),
                                          ("wvt", vp[:, t, :])):
                            ps = psum.tile([P, 512], f32, tag="psc", name="psc")
                            for kc in range(CB):
                                nc.tensor.matmul(
                                    ps[:, :], gnt[t][:, kc, pb * P:(pb + 1) * P],
                                    w_sb[w_nm][:, kc, :],
                                    start=(kc == 0), stop=(kc == CB - 1))
                            nc.scalar.copy(out=dst, in_=ps)

                    # scores sc[t,s] = sum_c q[t,c]k[s,c] (+ bqt.k[s], softmax-
                    # invariant terms dropped); SCALE folded into the exp.
                    # DVE does the batched muls; ACT reduces via accum_out.
                    sc = temp4.tile([P, T, T], f32, tag="sc", name="sc")
                    junkb = temp4.tile([P, C], bf16, tag="junkb", name="junkb")
                    with nc.allow_low_precision("bf16 score rounding ok"):
                        for t in range(T):
                            mbuf = temp4.tile([P, T, C], bf16, tag="mbuf",
                                              name="mbuf", bufs=3)
                            nc.vector.tensor_tensor(
                                out=mbuf, in0=kp,
                                in1=qp[:, t, :].unsqueeze(1).to_broadcast([P, T, C]),
                                op=MULT)
                            if (pb + t) % 4 == 3:
                                nc.vector.tensor_reduce(
                                    out=sc[:, t, :], in_=mbuf, axis=AX.X, op=ADD)
                            else:
                                for s in range(T):
                                    nc.scalar.activation(
                                        out=junkb, in_=mbuf[:, s, :], func=AF.Copy,
                                        accum_out=sc[:, t, :][:, s:s + 1])
                        mbufd = temp4.tile([P, T, C], bf16, tag="mbuf", name="mbufd", bufs=3)
                        nc.vector.tensor_tensor(
                            out=mbufd, in0=kp,
                            in1=bqt_bc.unsqueeze(1).to_broadcast([P, T, C]),
                            op=MULT)
                        dotk = temp4.tile([P, T], f32, tag="dotk", name="dotk")
                        nc.vector.tensor_reduce(out=dotk, in_=mbufd, axis=AX.X,
                                                op=ADD)
                        nc.vector.tensor_tensor(
                            out=sc, in0=sc,
                            in1=dotk.unsqueeze(1).to_broadcast([P, T, T]), op=ADD)
                    e5 = temp4.tile([P, T, T], f32, tag="e5", name="e5")
                    nc.scalar.activation(out=e5, in_=sc, func=AF.Exp, scale=SCALE)
                    den5 = temp4.tile([P, T], f32, tag="den5", name="den5")
                    nc.vector.tensor_reduce(out=den5, in_=e5, axis=AX.X, op=ADD)
                    rden5 = temp4.tile([P, T], f32, tag="rden5", name="rden5")
                    nc.vector.reciprocal(rden5, den5)
                    a5 = temp4.tile([P, T, T], bf16, tag="a5", name="a5")
                    with nc.allow_low_precision("bf16 att rounding ok"):
                        nc.vector.tensor_tensor(
                            out=a5, in0=e5,
                            in1=rden5.unsqueeze(2).to_broadcast([P, T, T]), op=MULT)

                    # htp[t] = sum_s att[t,s] * v5[s]: batched mul + add tree
                    with nc.allow_low_precision("bf16 htp rounding ok"):
                        for t in range(T):
                            eng = nc.gpsimd if (pb + t) % 2 == 1 else nc.vector
                            mb2 = temp4.tile([P, T, C], bf16, tag="mbuf2",
                                             name="mbuf2", bufs=2)
                            nc.vector.tensor_tensor(
                                out=mb2, in0=vp,
                                in1=a5[:, t, :].unsqueeze(2).to_broadcast([P, T, C]),
                                op=MULT)
                            h01 = temp4.tile([P, C], bf16, tag="h01", name="h01")
                            h23 = temp4.tile([P, C], bf16, tag="h23", name="h23")
                            eng.tensor_tensor(out=h01, in0=mb2[:, 0, :],
                                              in1=mb2[:, 1, :], op=ADD)
                            eng.tensor_tensor(out=h23, in0=mb2[:, 2, :],
                                              in1=mb2[:, 3, :], op=ADD)
                            eng.tensor_tensor(out=h01, in0=h01, in1=h23, op=ADD)
                            eng.tensor_tensor(out=htp_b[t][:, pb, :], in0=h01,
                                              in1=mb2[:, 4, :], op=ADD)

                for t in range(T):
                    htpT = temp2.tile([P, CB, HALF], bf16, tag="htpT", name="htpT")
                    for pb in range(QB):
                        nc.sync.dma_start(
                            out=htpT[:, :, pb * P:(pb + 1) * P],
                            in_=htp_b[t][:, pb, :], transpose=True)
                    # out = x + wot @ htpT + bot_eff
                    xh = temp2.tile([P, CB, HALF], f32, tag="xh", name="xh")
                    nc.sync.dma_start(
                        out=xh,
                        in_=x_d[t][:, 0:HALF].rearrange("(p j) hw -> p j hw", p=P))
                    out_sb = temp2.tile([P, CB, HALF], f32, tag="out_sb",
                                        name="out_sb")
                    for cb in range(CB):
                        ps = psum.tile([P, 512], f32, tag="psc", name="psc")
                        for kc in range(CB):
                            nc.tensor.matmul(
                                ps[:, :], w_sb["wot"][:, kc, cb * P:(cb + 1) * P],
                                htpT[:, kc, :],
                                start=(kc == 0), stop=(kc == CB - 1))
                        tmpo2 = temp2.tile([P, 512], f32, tag="tmpo2", name="tmpo2")
                        nc.vector.tensor_scalar_add(out=tmpo2, in0=ps,
                                                    scalar1=bias_sb["bot"][:, cb:cb + 1])
                        nc.gpsimd.tensor_tensor(out=out_sb[:, cb, :], in0=tmpo2,
                                                in1=xh[:, cb, :], op=ADD)
                    nc.sync.dma_start(
                        out=out_d[t].rearrange("(p j) hw -> p j hw", p=P),
                        in_=out_sb)

    nc.compile()
    return nc


# storage column s holds natural channel 4*(s % 128) + s // 128
_COL_PERM = np.array([4 * (s % P) + s // P for s in range(C)])


def _prepare_in_maps(inputs):
    import ml_dtypes
    x = np.asarray(inputs["x"], np.float32).reshape(B * T, C, HW)
    sel4 = np.zeros((P, G), np.float32)
    for p in range(P):
        sel4[p, p // 4] = 1.0
    bcast4 = sel4.T.copy()
    wT = {}
    for nm in ["wq", "wk", "wv", "wo", "wqt", "wkt", "wvt", "wot"]:
        w = np.asarray(inputs[nm], np.float32)   # [out, in]
        wt = w.T[:, _COL_PERM]                   # [in, out_perm]
        wT[nm] = np.ascontiguousarray(wt).astype(ml_dtypes.bfloat16)
    bot_eff = (np.asarray(inputs["bot"], np.float64)
               + np.asarray(inputs["wot"], np.float64)
               @ np.asarray(inputs["bvt"], np.float64)).astype(np.float32)
    common = {nm + "T": wT[nm] for nm in wT}
    for nm in ["bq", "bk", "bv", "bo"]:
        common[nm] = np.asarray(inputs[nm], np.float32)
    common["bot"] = bot_eff
    # bqt multiplies k-columns, which carry the permuted channel order
    common["bqt"] = np.asarray(inputs["bqt"], np.float32)[_COL_PERM] \
        .astype(ml_dtypes.bfloat16)
    for nm in ["gamma_s", "beta_s", "gamma_t", "beta_t"]:
        common[nm] = np.asarray(inputs[nm], np.float32)
    common["sel4"] = sel4
    common["bcast4"] = bcast4

    in_maps = []
    for v in range(B):
        xv = x[v * T:(v + 1) * T]
        for h in range(2):
            if h == 0:
                xc = xv
            else:
                xc = np.concatenate([xv[..., HALF:], xv[..., :HALF]], axis=-1)
            m = dict(common)
            m["x"] = np.ascontiguousarray(xc)
            in_maps.append(m)
    return in_maps


def _run(inputs, trace=False):
    from concourse import bass_utils
    if "nc" not in _CACHE:
        _CACHE["nc"] = _build()
    nc = _CACHE["nc"]
    in_maps = _prepare_in_maps(inputs)
    if trace:
        try:
            from antenv.axon_hooks import get_axon_ntff_profile_hook  # noqa: F401
        except ModuleNotFoundError:
            trace = False
    res = bass_utils.run_bass_kernel_spmd(nc, in_maps, core_ids=list(range(8)),
                                          trace=trace)
    out = np.empty((B * T, C, HW), np.float32)
    for v in range(B):
        for h in range(2):
            o = res.results[2 * v + h]["out"]  # [T, C, HALF]
            if h == 0:
                out[v * T:(v + 1) * T, :, :HALF] = o
            else:
                out[v * T:(v + 1) * T, :, HALF:] = o
    return out.reshape(B * T, C, 32, 32), res


def kernel(**inputs) -> np.ndarray:
    out, _ = _run(inputs, trace=False)
    return out



# revision 21
# speedup vs baseline: 1.4955x; 1.4955x over previous
"""Trainium2 Bass kernel for nn_AttnBlock_Spatio_Temporal (B=4,T=5,C=512,H=W=32).

Distribution: 8 cores = (video b in 0..3) x (pixel-half h in 0..1); host rolls
the HW axis per core so its own 512 pixels come first. All heavy matmuls run
in fp8e4 DoubleRow (K=256 per MM). Spatial attention computes scores
TRANSPOSED (keys on partitions) so no attention transpose is needed; softmax
normalization is deferred through the v- and wo-matmuls and applied once per
output pixel (PE broadcast of 1/den). GroupNorm_t cross-half stats use two
batched pair-AllReduces (frames 0-3 in one [128,2] op, frame 4 in [32,2]).

Channel-major layout: channel c lives at (partition p, block j), c = 4p + j.
Weight matrices are host-permuted (columns) and pre-scaled x16 into fp8.
"""
import numpy as np

B, T, C, HW = 4, 5, 512, 1024
G = 32
EPS = 1e-6
P = 128
CB = C // P          # 4 channel blocks
HALF = HW // 2       # 512 own pixels
KB = HW // P         # 8 key-pixel blocks
QB = HALF // P       # 4 query/pixel blocks
SCALE = float(C) ** -0.5
INV_CNT = 1.0 / 16384.0   # per-group element count (16ch * 1024px)

_CACHE = {}


def _build():
    import concourse.bacc as bacc
    import concourse.tile as tile
    import concourse.mybir as mybir

    f32 = mybir.dt.float32
    bf16 = mybir.dt.bfloat16
    fp8 = mybir.dt.float8e4
    MULT = mybir.AluOpType.mult
    ADD = mybir.AluOpType.add
    SUB = mybir.AluOpType.subtract
    AF = mybir.ActivationFunctionType
    AX = mybir.AxisListType
    DR = mybir.MatmulPerfMode.DoubleRow

    nc = bacc.Bacc("TRN2", target_bir_lowering=False, debug=False, num_devices=8)

    x_d = nc.dram_tensor("x", [T, C, HW], f32, kind="ExternalInput").ap()
    fp8_w = ["wq", "wk", "wv", "wo", "wqt", "wkt", "wvt"]
    w_d = {nm: nc.dram_tensor(nm + "T", [C, C], fp8, kind="ExternalInput").ap()
           for nm in fp8_w}
    wot_d = nc.dram_tensor("wotT", [C, C], bf16, kind="ExternalInput").ap()
    boh_d = nc.dram_tensor("bo_half", [C], bf16, kind="ExternalInput").ap()
    bot4_d = nc.dram_tensor("bot4", [C], bf16, kind="ExternalInput").ap()
    g_d = {nm: nc.dram_tensor(nm, [C], f32, kind="ExternalInput").ap()
           for nm in ["gamma_s", "beta_s16", "gamma_t", "beta_t16"]}
    sel_d = nc.dram_tensor("sel4", [P, G], f32, kind="ExternalInput").ap()
    bc16_d = nc.dram_tensor("bcast16", [G, P], f32, kind="ExternalInput").ap()
    bc16f_d = nc.dram_tensor("bcast16f", [P, 4, P], f32,
                             kind="ExternalInput").ap()
    ind5_d = nc.dram_tensor("ind5", [25, 5], bf16, kind="ExternalInput").ap()
    sel25_d = nc.dram_tensor("sel25", [P, 25, 25], bf16,
                             kind="ExternalInput").ap()
    out_d = nc.dram_tensor("out", [T, C, HALF], f32, kind="ExternalOutput").ap()

    def cpart(ap_1d):  # [C] dram -> [128, CB] tile order (c = 4p + j)
        return ap_1d.rearrange("(p j) -> p j", p=P)

    with tile.TileContext(nc) as tc, \
         nc.allow_low_precision("fp8/bf16 pipeline by design"):
        with tc.tile_pool(name="consts", bufs=1) as consts, \
             tc.tile_pool(name="stat", bufs=2) as stat, \
             tc.tile_pool(name="spatio_p", bufs=T) as spatio_p, \
             tc.tile_pool(name="gnt_p", bufs=T) as gnt_p, \
             tc.tile_pool(name="psA", bufs=3, space="PSUM") as psA, \
             tc.tile_pool(name="psS", bufs=2, space="PSUM") as psS, \
             tc.tile_pool(name="dram", bufs=4, space="DRAM") as dram:

            # ---------------- constants ----------------
            spat_cm = tc.tile_pool(name="spat", bufs=1)
            spat = spat_cm.__enter__()
            nc.sync.dma_start(
                out=(xf0 := spat.tile([P, CB, HW], f32, tag="xf", name="xf0",
                                      bufs=2)),
                in_=x_d[0].rearrange("(p j) hw -> p j hw", p=P))
            w_sb = {}
            for nm in ["wk", "wq", "wv"]:
                w_sb[nm] = consts.tile([P, CB, C], fp8, tag="w_" + nm,
                                       name="w_" + nm)
                nc.sync.dma_start(
                    out=w_sb[nm],
                    in_=w_d[nm].rearrange("(p kc) co -> p kc co", p=P))
            gam_sb = {}
            for nm in ["gamma_s", "beta_s16", "gamma_t", "beta_t16"]:
                gam_sb[nm] = consts.tile([P, CB], f32, tag="g_" + nm,
                                         name="g_" + nm)
                nc.sync.dma_start(out=gam_sb[nm], in_=cpart(g_d[nm]))
            sel4 = consts.tile([P, G], f32, tag="sel4", name="sel4")
            nc.sync.dma_start(out=sel4, in_=sel_d)
            bc16 = consts.tile([G, P], f32, tag="bc16", name="bc16")
            nc.sync.dma_start(out=bc16, in_=bc16_d)
            bc16f = consts.tile([P, 4, P], f32, tag="bc16f", name="bc16f")
            nc.sync.dma_start(out=bc16f, in_=bc16f_d)
            eps128 = consts.tile([P, 1], f32, tag="eps", name="eps128")
            nc.vector.memset(eps128, EPS)
            ones8 = consts.tile([P, 2, 1], fp8, tag="ones8", name="ones8")
            nc.vector.memset(ones8, 1.0)
            onesrow = consts.tile([1, 512], bf16, tag="onesrow", name="onesrow")
            nc.vector.memset(onesrow, 1.0)
            twos = consts.tile([1, P], bf16, tag="twos", name="twos")
            nc.vector.memset(twos, 2.0)
            ones1 = consts.tile([P, 1], bf16, tag="ones1", name="ones1")
            nc.vector.memset(ones1, 1.0)
            boh = consts.tile([1, C], bf16, tag="boh", name="boh")
            nc.sync.dma_start(out=boh, in_=boh_d.unsqueeze(0))
            bot4 = consts.tile([1, C], bf16, tag="bot4", name="bot4")
            nc.sync.dma_start(out=bot4, in_=bot4_d.unsqueeze(0))
            ind5 = consts.tile([25, 5], bf16, tag="ind5", name="ind5")
            nc.sync.dma_start(out=ind5, in_=ind5_d)
            sel25 = consts.tile([P, 25, 25], bf16, tag="sel25", name="sel25")
            nc.sync.dma_start(out=sel25, in_=sel25_d)

            # ---------- GroupNorm helpers ----------
            def affine_rg(g2, rows):
                """g2: [rows,2] SBUF (sum,sumsq) -> rg [rows,2] =
                (rstd, -mu*rstd)."""
                m2 = stat.tile([P, 2], f32, tag="m2", name="m2")[0:rows, :]
                nc.scalar.activation(out=m2, in_=g2, func=AF.Copy,
                                     scale=INV_CNT)
                rg = stat.tile([P, 2], f32, tag="rg", name="rg")[0:rows, :]
                nc.vector.tensor_tensor(out=rg[:, 0:1], in0=m2[:, 0:1],
                                        in1=m2[:, 0:1], op=MULT)
                nc.vector.tensor_tensor(out=rg[:, 0:1], in0=m2[:, 1:2],
                                        in1=rg[:, 0:1], op=SUB)
                nc.scalar.activation(out=rg[:, 0:1], in_=rg[:, 0:1],
                                     func=AF.Sqrt, bias=eps128[0:rows, :],
                                     scale=1.0)
                nc.vector.reciprocal(rg[:, 0:1], rg[:, 0:1])
                nc.vector.tensor_scalar(out=rg[:, 1:2], in0=m2[:, 0:1],
                                        scalar1=rg[:, 0:1], scalar2=-1.0,
                                        op0=MULT, op1=MULT)
                return rg

            def affine_apply(rg32, gamma, beta16, tag, lhsT=None):
                """rg32: [rows,2] at base partition 0 -> scale/shift [P, CB]."""
                ps_bc = psS.tile([P, 512], f32, tag="sm", name="psbc_" + tag)
                nc.tensor.matmul(ps_bc[:, 0:2], bc16 if lhsT is None else lhsT,
                                 rg32, start=True, stop=True)
                sc = stat.tile([P, CB], f32, tag="sc" + tag, name="sc" + tag)
                sh = stat.tile([P, CB], f32, tag="sh" + tag, name="sh" + tag)
                nc.vector.tensor_scalar_mul(out=sc, in0=gamma,
                                            scalar1=ps_bc[:, 0:1])
                nc.vector.scalar_tensor_tensor(out=sh, in0=gamma,
                                               scalar=ps_bc[:, 1:2],
                                               in1=beta16, op0=MULT, op1=ADD)
                return sc, sh

            def gn_stats(src, is_f32, sums_name):
                """per-channel (sum, sumsq) of [P, CB, n] -> sums [P, CB, 2]"""
                sums = spat.tile([P, CB, 2], f32, tag="sums", name=sums_name,
                                 bufs=2)
                nc.vector.tensor_reduce(out=sums[:, :, 0:1], in_=src,
                                        axis=AX.X, op=ADD)
                n = src.shape[2]
                sq = spat.tile([P, HW], bf16, tag="sqj", name="sq_" + sums_name,
                               bufs=2)
                for j in range(CB):
                    nc.scalar.activation(out=sq[:, 0:n], in_=src[:, j, :],
                                         func=AF.Square,
                                         accum_out=sums[:, j, 1:2])
                return sums

            def group_sums(sums, out_rows):
                """sums [P,CB,2] -> write [32,2] group sums into out_rows."""
                ps_g = psS.tile([P, 512], f32, tag="sm", name="psg")
                nc.tensor.matmul(ps_g[0:G, 0:2 * CB], sel4,
                                 sums.rearrange("p j s -> p (j s)"),
                                 start=True, stop=True)
                nc.vector.tensor_reduce(
                    out=out_rows,
                    in_=ps_g[0:G, 0:2 * CB].rearrange("g (j s) -> g s j", s=2),
                    axis=AX.X, op=ADD)

            xfs = [None] * T
            xfs[0] = xf0
            hns = [None] * T
            spatio_tiles = [None] * T
            gnt = [None] * T
            g2b4 = stat.tile([P, 2], f32, tag="g2b4", name="g2b4", bufs=1)
            g2f4 = stat.tile([G, 2], f32, tag="g2f4", name="g2f4", bufs=1)

            def gn_s(f):
                sums = gn_stats(xfs[f], True, f"sums_s{f}")
                g2s = stat.tile([G, 2], f32, tag="g2s", name=f"g2s{f}")
                group_sums(sums, g2s)
                rg = affine_rg(g2s, G)
                return affine_apply(rg, gam_sb["gamma_s"], gam_sb["beta_s16"],
                                    "s")

            def hn_make(f, sc, sh):
                hn = spat.tile([P, CB, HW], fp8, tag="hn", name=f"hn{f}",
                               bufs=2)
                for j in range(CB):
                    nc.vector.tensor_scalar(out=hn[:, j, :], in0=xfs[f][:, j, :],
                                            scalar1=sc[:, j:j + 1],
                                            scalar2=sh[:, j:j + 1],
                                            op0=MULT, op1=ADD)
                hns[f] = hn

            def load_x(f):
                xf = spat.tile([P, CB, HW], f32, tag="xf", name=f"xf{f}",
                               bufs=2)
                nc.sync.dma_start(
                    out=xf, in_=x_d[f].rearrange("(p j) hw -> p j hw", p=P))
                xfs[f] = xf

            # ================= spatial phase =================
            sc_s, sh_s = gn_s(0)
            for nm in ["wo", "wqt", "wkt", "wvt"]:
                w_sb[nm] = consts.tile([P, CB, C], fp8, tag="w_" + nm,
                                       name="w_" + nm)
                nc.sync.dma_start(
                    out=w_sb[nm],
                    in_=w_d[nm].rearrange("(p kc) co -> p kc co", p=P))
            wot_sb = consts.tile([P, CB, C], bf16, tag="w_wot", name="w_wot")
            nc.sync.dma_start(
                out=wot_sb, in_=wot_d.rearrange("(p kc) co -> p kc co", p=P))
            hn_make(0, sc_s, sh_s)

            for f in range(T):
                hn = hns[f]
                if f + 1 < T:
                    load_x(f + 1)

                # ---- k conv: [P, CB, HW] fp8 = 4*k ----
                k_sb = spat.tile([P, CB, HW], fp8, tag="k_sb", name="k_sb")
                for jp in (0, 2):
                    for h in (0, 1):
                        ps = psA.tile([P, 2, 512], f32, tag="big", name="psk")
                        for i in (0, 1):
                            for kcp in (0, 1):
                                nc.tensor.matmul(
                                    ps[:, i, :],
                                    w_sb["wk"][:, 2 * kcp:2 * kcp + 2,
                                               (jp + i) * P:(jp + i + 1) * P],
                                    hn[:, 2 * kcp:2 * kcp + 2,
                                       h * 512:(h + 1) * 512],
                                    start=(kcp == 0), stop=(kcp == 1),
                                    perf_mode=DR)
                        nc.scalar.activation(
                            out=k_sb[:, jp:jp + 2, h * 512:(h + 1) * 512],
                            in_=ps, func=AF.Copy, scale=1.0 / 64.0)

                # ---- vT conv: [P(kpix), KB, C] fp8 = 4*v ----
                vT = spat.tile([P, KB, C], fp8, tag="vT", name="vT")
                for pp in (0, 2, 4, 6):
                    ps = psA.tile([P, 2, 512], f32, tag="big", name="psv")
                    for i in (0, 1):
                        for kcp in (0, 1):
                            nc.tensor.matmul(
                                ps[:, i, :],
                                hn[:, 2 * kcp:2 * kcp + 2,
                                   (pp + i) * P:(pp + i + 1) * P],
                                w_sb["wv"][:, 2 * kcp:2 * kcp + 2, :],
                                start=(kcp == 0), stop=(kcp == 1),
                                perf_mode=DR)
                    nc.vector.tensor_scalar(out=vT[:, pp:pp + 2, :], in0=ps,
                                            scalar1=1.0 / 64.0, scalar2=0.0,
                                            op0=MULT, op1=ADD)

                # stats for next frame overlap the conv matmuls
                if f + 1 < T:
                    sc_s, sh_s = gn_s(f + 1)

                # ---- q conv: [P, CB, HALF] fp8 = 4*q ----
                q_sb = spat.tile([P, CB, HALF], fp8, tag="q_sb", name="q_sb")
                for jp in (0, 2):
                    ps = psA.tile([P, 2, 512], f32, tag="big", name="psq")
                    for i in (0, 1):
                        for kcp in (0, 1):
                            nc.tensor.matmul(
                                ps[:, i, :],
                                w_sb["wq"][:, 2 * kcp:2 * kcp + 2,
                                           (jp + i) * P:(jp + i + 1) * P],
                                hn[:, 2 * kcp:2 * kcp + 2, 0:HALF],
                                start=(kcp == 0), stop=(kcp == 1),
                                perf_mode=DR)
                    nc.scalar.activation(out=q_sb[:, jp:jp + 2, :], in_=ps,
                                         func=AF.Copy, scale=1.0 / 64.0)

                if f + 1 < T:
                    hn_make(f + 1, sc_s, sh_s)

                # ---- scoresT + exp: expT [P(kpix), KB, HALF] fp8 ----
                expT = spat.tile([P, KB, HALF], fp8, tag="expT", name="expT")
                for kp in (0, 2, 4, 6):
                    ps = psA.tile([P, 2, 512], f32, tag="big", name="pssc")
                    for i in (0, 1):
                        for kcp in (0, 1):
                            nc.tensor.matmul(
                                ps[:, i, :],
                                k_sb[:, 2 * kcp:2 * kcp + 2,
                                     (kp + i) * P:(kp + i + 1) * P],
                                q_sb[:, 2 * kcp:2 * kcp + 2, :],
                                start=(kcp == 0), stop=(kcp == 1),
                                perf_mode=DR)
                    nc.scalar.activation(out=expT[:, kp:kp + 2, :], in_=ps,
                                         func=AF.Exp, scale=SCALE / 16.0)

                # ---- den = sum_k exp ; rdenB = 2/den bcast ----
                ps_den = psS.tile([P, 512], f32, tag="sm", name="psden")
                for i in range(4):
                    nc.tensor.matmul(ps_den[0:1, :], ones8,
                                     expT[:, 2 * i:2 * i + 2, :],
                                     start=(i == 0), stop=(i == 3),
                                     perf_mode=DR)
                den_sb = spat.tile([1, 512], bf16, tag="den_sb", name="den_sb")
                nc.scalar.activation(out=den_sb, in_=ps_den[0:1, :],
                                     func=AF.Copy, scale=1.0)
                rden = spat.tile([1, 512], bf16, tag="rden", name="rden")
                nc.vector.reciprocal(rden, ps_den[0:1, :])
                ps_rb = psS.tile([P, 512], f32, tag="sm", name="psrb")
                nc.tensor.matmul(ps_rb[:, :], twos, rden, start=True, stop=True)
                rdenB = spat.tile([P, 512], bf16, tag="rdenB", name="rdenB")
                nc.scalar.activation(out=rdenB, in_=ps_rb, func=AF.Copy,
                                     scale=1.0)

                # ---- hsp = vT'.T @ expT : [P, CB, HALF] fp8 = hsp_u/32 ----
                hsp = spat.tile([P, CB, HALF], fp8, tag="hsp", name="hsp")
                for cp in (0, 2):
                    ps = psA.tile([P, 2, 512], f32, tag="big", name="psh")
                    for i in (0, 1):
                        for kbp in range(4):
                            nc.tensor.matmul(
                                ps[:, i, :],
                                vT[:, 2 * kbp:2 * kbp + 2,
                                   (cp + i) * P:(cp + i + 1) * P],
                                expT[:, 2 * kbp:2 * kbp + 2, :],
                                start=(kbp == 0), stop=(kbp == 3),
                                perf_mode=DR)
                    nc.vector.tensor_scalar(out=hsp[:, cp:cp + 2, :], in0=ps,
                                            scalar1=1.0 / 128.0, scalar2=0.0,
                                            op0=MULT, op1=ADD)

                # ---- wo conv + deferred normalize + residual ----
                spatio = spatio_p.tile([P, CB, HALF], bf16, tag="spatio",
                                       name=f"spatio{f}")
                for cp in (0, 2):
                    ps = psA.tile([P, 2, 512], f32, tag="big", name="pso")
                    for i in (0, 1):
                        for kcp in (0, 1):
                            nc.tensor.matmul(
                                ps[:, i, :],
                                w_sb["wo"][:, 2 * kcp:2 * kcp + 2,
                                           (cp + i) * P:(cp + i + 1) * P],
                                hsp[:, 2 * kcp:2 * kcp + 2, :],
                                start=(kcp == 0), stop=False, perf_mode=DR)
                        nc.tensor.matmul(
                            ps[:, i, :],
                            boh[0:1, (cp + i) * P:(cp + i + 1) * P],
                            den_sb, start=False, stop=True)
                    s_n = spat.tile([P, 2, 512], bf16, tag="s_n", name="s_n",
                                    bufs=2)
                    nc.vector.tensor_tensor(
                        out=s_n, in0=ps,
                        in1=rdenB.unsqueeze(1).to_broadcast([P, 2, 512]),
                        op=MULT)
                    nc.gpsimd.tensor_tensor(out=spatio[:, cp:cp + 2, :],
                                            in0=s_n,
                                            in1=xfs[f][:, cp:cp + 2, 0:512],
                                            op=ADD)

                spatio_tiles[f] = spatio

                # ---- GN_t partial stats ----
                sums_t = gn_stats(spatio, False, f"sums_t{f}")
                if f < 4:
                    group_sums(sums_t, g2b4[f * G:(f + 1) * G, :])
                else:
                    group_sums(sums_t, g2f4)

                if f == 3:
                    bnc_in4 = dram.tile([P, 2], f32, tag="bnc_in4",
                                        name="bnc_in4")
                    bnc_out4 = dram.tile([P, 2], f32, tag="bnc_out4",
                                         name="bnc_out4")
                    nc.sync.dma_start(out=bnc_in4[:], in_=g2b4[:])
                    nc.gpsimd.collective_compute(
                        "AllReduce", ADD,
                        replica_groups=[[0, 1], [2, 3], [4, 5], [6, 7]],
                        ins=[bnc_in4.opt()], outs=[bnc_out4.opt()])

            # frame-4 collective
            bnc_in1 = dram.tile([G, 2], f32, tag="bnc_in1", name="bnc_in1")
            bnc_out1 = dram.tile([G, 2], f32, tag="bnc_out1", name="bnc_out1")
            nc.sync.dma_start(out=bnc_in1[:], in_=g2f4[:])
            nc.gpsimd.collective_compute(
                "AllReduce", ADD,
                replica_groups=[[0, 1], [2, 3], [4, 5], [6, 7]],
                ins=[bnc_in1.opt()], outs=[bnc_out1.opt()])

            # ---- temporal GN affines (frames 0-3 batched) + gnt ----
            spat_cm.__exit__(None, None, None)
            tempo_cm = tc.tile_pool(name="tempo", bufs=1)
            tempo = tempo_cm.__enter__()
            gsum4 = stat.tile([P, 2], f32, tag="gsum4", name="gsum4", bufs=1)
            nc.sync.dma_start(out=gsum4[:], in_=bnc_out4[:])
            rg4 = affine_rg(gsum4, P)

            def gnt_make(f, rg32, lhsT=None):
                sc, sh = affine_apply(rg32, gam_sb["gamma_t"],
                                      gam_sb["beta_t16"], "t", lhsT=lhsT)
                g = gnt_p.tile([P, CB, HALF], fp8, tag="gnt", name=f"gnt{f}")
                for j in range(CB):
                    nc.vector.tensor_scalar(out=g[:, j, :],
                                            in0=spatio_tiles[f][:, j, :],
                                            scalar1=sc[:, j:j + 1],
                                            scalar2=sh[:, j:j + 1],
                                            op0=MULT, op1=ADD)
                gnt[f] = g

            # ================= temporal phase =================
            q5c = tempo.tile([P, T, CB, 512], bf16, tag="q5c", name="q5c")
            k5c = tempo.tile([P, T, CB, 512], bf16, tag="k5c", name="k5c")
            v5P = tempo.tile([P, QB, T, 512], fp8, tag="v5P", name="v5P")

            def tconv(t):
                for nm, dst in (("wqt", q5c), ("wkt", k5c)):
                    for jp in (0, 2):
                        ps = psA.tile([P, 2, 512], f32, tag="big", name="pst")
                        for i in (0, 1):
                            for kcp in (0, 1):
                                nc.tensor.matmul(
                                    ps[:, i, :],
                                    w_sb[nm][:, 2 * kcp:2 * kcp + 2,
                                             (jp + i) * P:(jp + i + 1) * P],
                                    gnt[t][:, 2 * kcp:2 * kcp + 2, :],
                                    start=(kcp == 0), stop=(kcp == 1),
                                    perf_mode=DR)
                        nc.scalar.activation(out=dst[:, t, jp:jp + 2, :],
                                             in_=ps, func=AF.Copy,
                                             scale=1.0 / 64.0)
                for pp in (0, 2):
                    ps = psA.tile([P, 2, 512], f32, tag="big", name="pstv")
                    for i in (0, 1):
                        for kcp in (0, 1):
                            nc.tensor.matmul(
                                ps[:, i, :],
                                gnt[t][:, 2 * kcp:2 * kcp + 2,
                                       (pp + i) * P:(pp + i + 1) * P],
                                w_sb["wvt"][:, 2 * kcp:2 * kcp + 2, :],
                                start=(kcp == 0), stop=(kcp == 1),
                                perf_mode=DR)
                    nc.vector.tensor_scalar(out=v5P[:, pp:pp + 2, t, :],
                                            in0=ps, scalar1=1.0 / 64.0,
                                            scalar2=0.0, op0=MULT, op1=ADD)

            for f in range(4):
                gnt_make(f, rg4, lhsT=bc16f[:, f, :])
            for t in range(4):
                tconv(t)
            gsum1 = stat.tile([G, 2], f32, tag="gsum1", name="gsum1", bufs=1)
            nc.sync.dma_start(out=gsum1[:], in_=bnc_out1[:])
            rg1 = affine_rg(gsum1, G)
            gnt_make(4, rg1)
            tconv(4)

            # ---- 5x5 per-pixel scores on PE via em + ones-contraction ----
            ps_sc = psS.tile([P, 512], f32, tag="sm", name="ps_sc")
            for t in range(T):
                for s in range(T):
                    em = tempo.tile([P, CB, 512], bf16, tag="em", name="em",
                                    bufs=2)
                    nc.vector.tensor_tensor(out=em, in0=q5c[:, t, :, :],
                                            in1=k5c[:, s, :, :], op=MULT)
                    ts_ = 5 * t + s
                    for kc in range(CB):
                        nc.tensor.matmul(
                            ps_sc[0:25, :], sel25[:, ts_, :],
                            em[:, kc, :],
                            start=(ts_ == 0 and kc == 0),
                            stop=(ts_ == 24 and kc == 3))
            exp_sb = tempo.tile([32, 512], bf16, tag="exp_sb", name="exp_sb")
            nc.vector.memset(exp_sb[25:32, :], 0.0)
            nc.scalar.activation(out=exp_sb[0:25, :], in_=ps_sc[0:25, :],
                                 func=AF.Exp, scale=SCALE / 16.0)
            ps_d5 = psS.tile([P, 512], f32, tag="sm", name="ps_d5")
            nc.tensor.matmul(ps_d5[0:5, :], ind5, exp_sb[0:25, :], start=True,
                             stop=True)
            rden5 = tempo.tile([16, 512], bf16, tag="rden5", name="rden5")
            nc.vector.memset(rden5[5:16, :], 0.0)
            nc.vector.reciprocal(rden5[0:5, :], ps_d5[0:5, :])
            a5P = tempo.tile([P, QB, 32], bf16, tag="a5P", name="a5P")
            nc.sync.dma_start(out=a5P, in_=exp_sb, transpose=True)
            rdenP = tempo.tile([P, QB, 16], bf16, tag="rdenP", name="rdenP")
            nc.sync.dma_start(out=rdenP, in_=rden5, transpose=True)
            a5n = tempo.tile([P, QB, 25], f32, tag="a5n", name="a5n")
            nc.vector.tensor_tensor(
                out=a5n.rearrange("p q (t s) -> p q t s", t=5),
                in0=a5P[:, :, 0:25].rearrange("p q (t s) -> p q t s", t=5),
                in1=rdenP[:, :, 0:5].unsqueeze(3).to_broadcast([P, QB, 5, 5]),
                op=MULT)

            # ---- htp + wot + residual out, frame by frame ----
            xh0 = tempo.tile([P, CB, HALF], f32, tag="xh", name="xh0", bufs=2)
            nc.sync.dma_start(
                out=xh0, in_=x_d[0][:, 0:HALF].rearrange("(p j) hw -> p j hw",
                                                         p=P))
            xhs = [xh0, None, None, None, None]
            for t in range(T):
                if t + 1 < T:
                    xh = tempo.tile([P, CB, HALF], f32, tag="xh",
                                    name=f"xh{t + 1}", bufs=2)
                    nc.sync.dma_start(
                        out=xh,
                        in_=x_d[t + 1][:, 0:HALF].rearrange(
                            "(p j) hw -> p j hw", p=P))
                    xhs[t + 1] = xh
                htpT = tempo.tile([P, CB, HALF], bf16, tag="htpT", name="htpT")
                for pb in range(QB):
                    mb = tempo.tile([P, T, 512], bf16, tag="mb", name="mb",
                                    bufs=2)
                    for s in range(T):
                        nc.vector.tensor_scalar_mul(
                            out=mb[:, s, :], in0=v5P[:, pb, s, :],
                            scalar1=a5n[:, pb, 5 * t + s:5 * t + s + 1])
                    nc.gpsimd.tensor_tensor(out=mb[:, 0:2, :],
                                            in0=mb[:, 0:2, :],
                                            in1=mb[:, 2:4, :], op=ADD)
                    t2 = tempo.tile([P, 512], bf16, tag="t2", name="t2",
                                    bufs=2)
                    nc.vector.tensor_tensor(out=t2, in0=mb[:, 0, :],
                                            in1=mb[:, 1, :], op=ADD)
                    htpP = tempo.tile([P, 512], bf16, tag="htpP", name="htpP",
                                      bufs=2)
                    nc.vector.tensor_tensor(out=htpP, in0=t2, in1=mb[:, 4, :],
                                            op=ADD)
                    nc.sync.dma_start(out=htpT[:, :, pb * P:(pb + 1) * P],
                                      in_=htpP, transpose=True)
                for cp in (0, 2):
                    ps = psA.tile([P, 2, 512], f32, tag="big", name="psw")
                    for i in (0, 1):
                        for kc in range(CB):
                            nc.tensor.matmul(
                                ps[:, i, :],
                                wot_sb[:, kc, (cp + i) * P:(cp + i + 1) * P],
                                htpT[:, kc, :],
                                start=(kc == 0), stop=False)
                        nc.tensor.matmul(
                            ps[:, i, :],
                            bot4[0:1, (cp + i) * P:(cp + i + 1) * P],
                            onesrow, start=False, stop=True)
                    oc = tempo.tile([P, 2, 512], f32, tag="oc", name="oc",
                                    bufs=2)
                    nc.vector.scalar_tensor_tensor(
                        out=oc, in0=ps, scalar=0.25,
                        in1=xhs[t][:, cp:cp + 2, :], op0=MULT, op1=ADD)
                    nc.sync.dma_start(
                        out=out_d[t].rearrange("(p j) hw -> p j hw",
                                               p=P)[:, cp:cp + 2, :],
                        in_=oc)
            tempo_cm.__exit__(None, None, None)

    nc.compile()
    return nc


# storage column s holds natural channel 4*(s % 128) + s // 128
_COL_PERM = np.array([4 * (s % P) + s // P for s in range(C)])


def _prepare_in_maps(inputs):
    import ml_dtypes
    fp8 = ml_dtypes.float8_e4m3
    x = np.asarray(inputs["x"], np.float32).reshape(B * T, C, HW)
    sel4 = np.zeros((P, G), np.float32)
    for p in range(P):
        sel4[p, p // 4] = 1.0
    bcast16 = sel4.T.copy() * 16.0
    common = {}
    for nm in ["wq", "wk", "wv", "wo", "wqt", "wkt", "wvt"]:
        w = np.asarray(inputs[nm], np.float32)   # [out, in]
        wt = np.ascontiguousarray(w.T[:, _COL_PERM]) * 16.0
        common[nm + "T"] = np.clip(wt, -240.0, 240.0).astype(fp8)
    wot = np.asarray(inputs["wot"], np.float32)
    common["wotT"] = np.ascontiguousarray(wot.T[:, _COL_PERM]).astype(
        ml_dtypes.bfloat16)
    bo_eff = (np.asarray(inputs["bo"], np.float64)
              + np.asarray(inputs["wo"], np.float64)
              @ np.asarray(inputs["bv"], np.float64))
    common["bo_half"] = (0.5 * bo_eff[_COL_PERM]).astype(ml_dtypes.bfloat16)
    bot_eff = (np.asarray(inputs["bot"], np.float64)
               + np.asarray(inputs["wot"], np.float64)
               @ np.asarray(inputs["bvt"], np.float64))
    common["bot4"] = (4.0 * bot_eff[_COL_PERM]).astype(ml_dtypes.bfloat16)
    common["gamma_s"] = np.asarray(inputs["gamma_s"], np.float32)
    common["beta_s16"] = 16.0 * np.asarray(inputs["beta_s"], np.float32)
    common["gamma_t"] = np.asarray(inputs["gamma_t"], np.float32)
    common["beta_t16"] = 16.0 * np.asarray(inputs["beta_t"], np.float32)
    common["sel4"] = sel4
    common["bcast16"] = bcast16
    bc16f = np.zeros((P, 4, P), np.float32)
    for f in range(4):
        for p in range(P):
            bc16f[f * G + p // 4, f, p] = 16.0
    common["bcast16f"] = bc16f
    ind5 = np.zeros((25, 5), np.float32)
    for t in range(5):
        for s in range(5):
            ind5[5 * t + s, t] = 1.0
    common["ind5"] = ind5.astype(ml_dtypes.bfloat16)
    sel25 = np.zeros((P, 25, 25), np.float32)
    for ts_ in range(25):
        sel25[:, ts_, ts_] = 1.0
    common["sel25"] = sel25.astype(ml_dtypes.bfloat16)

    in_maps = []
    for v in range(B):
        xv = x[v * T:(v + 1) * T]
        for h in range(2):
            if h == 0:
                xc = xv
            else:
                xc = np.concatenate([xv[..., HALF:], xv[..., :HALF]], axis=-1)
            m = dict(common)
            m["x"] = np.ascontiguousarray(xc)
            in_maps.append(m)
    return in_maps


def _run(inputs, trace=False):
    from concourse import bass_utils
    if "nc" not in _CACHE:
        _CACHE["nc"] = _build()
    nc = _CACHE["nc"]
    in_maps = _prepare_in_maps(inputs)
    if trace:
        try:
            from antenv.axon_hooks import get_axon_ntff_profile_hook  # noqa: F401
        except ModuleNotFoundError:
            trace = False
    res = bass_utils.run_bass_kernel_spmd(nc, in_maps, core_ids=list(range(8)),
                                          trace=trace)
    out = np.empty((B * T, C, HW), np.float32)
    for v in range(B):
        for h in range(2):
            o = res.results[2 * v + h]["out"]  # [T, C, HALF]
            if h == 0:
                out[v * T:(v + 1) * T, :, :HALF] = o
            else:
                out[v * T:(v + 1) * T, :, HALF:] = o
    return out.reshape(B * T, C, 32, 32), res


def kernel(**inputs) -> np.ndarray:
    out, _ = _run(inputs, trace=False)
    return out
